# revision 20
# baseline (speedup 1.0000x reference)
"""Trainium2 Bass kernel for nn_ChaoticDecoder.

Math: in the reference, attention scores are softmax(feat @ Wa + ba, axis=seq)
with feat = [x, ht_rep, ct_rep].  The ht/ct/bias contributions are constant
along the seq axis, so they cancel inside the softmax.  Hence

    alpha   = softmax(x @ Wa[:H], axis=seq)          (time-invariant!)
    context = sum_s alpha * x                        (time-invariant)
    G0      = context @ Wi + b                       (time-invariant)
    gates_t = G0 + h_t @ Wh                          (the only per-step matmul)

which turns the 52-GFLOP reference into ~1.4 GFLOP: a one-time attention
precompute plus a 64-step LSTM recurrence on (bs, 256) state.

Sharding: pure data-parallel over batch (32 -> 4 per core, 8 cores), weights
replicated, no collectives; the host concatenates the 8 per-core (4,1) outputs.

Device layout (everything transposed): hidden dim on partitions, batch on the
free dim.  gates live as packed PSUM tiles [partition = h%128,
free = (gate-block j, batch b)]; h_t^T slices are directly the matmul rhs for
the next step -- no per-step transposes anywhere.

Per-step critical-path structure:
  - gate columns host-permuted to [g, f, i, o]; the g block accumulates in
    its own PSUM bank so tanh(g) issues after only 5 matmuls (Tile deps are
    per-tile), overlapping the f/i/o matmuls.
  - one sigmoid ACT covers [f|i|o]; [sig f|sig i] multiplies the adjacent
    [ct|tanh g] state pair in a single (128, 16) DVE op.
  - G0 is injected into PSUM by the first matmuls of each step
    (lhsT = G0^T slices in fp16, rhs = identity, start=True): no DVE add.
  - a warm-up burst of dummy matmuls trips the PE HAM clock gate to 2.4 GHz
    before the recurrence; per-step PE gaps are < the 3.4 us re-throttle
    window, so it stays warm.
"""

import numpy as np

import concourse.bass as bass
import concourse.bacc as bacc
import concourse.mybir as mybir
import concourse.tile as tile
from concourse.bass_utils import run_bass_kernel_spmd
from concourse.masks import make_identity

BS, SEQ, H, OUT = 32, 64, 256, 1
NCORES = 8
B = BS // NCORES          # batch per core = 4
F32 = mybir.dt.float32

# Recurrence matmul dtype: float16 keeps 1 cycle/row PE speed with ~2e-4 final
# rel err; float32 is exact but ~2x slower per step.
REC_DT = mybir.dt.float16
REC_NP = np.float16

# gate-block order on device: [g g f f i i o o] (128-wide blocks of the 4H
# gate dim); host permutes Wh/Wi/b columns to match.
GATE_PERM = [4, 5, 2, 3, 0, 1, 6, 7]   # original block order: i i f f g g o o

N_WARM_MM = 1             # absorbs the gpsimd wait before the first transpose;
                          # phases 2-4's own matmuls warm the HAM clock gate


def _build_nc():
    nc = bacc.Bacc()

    xt32f = nc.declare_dram_parameter("xt32f", [H, B * SEQ], F32, isOutput=False)
    xt16f = nc.declare_dram_parameter("xt16f", [H, B * SEQ], REC_DT, isOutput=False)
    wax = nc.declare_dram_parameter("wax", [H, H], REC_DT, isOutput=False)
    wh = nc.declare_dram_parameter("wh", [H, 4 * H], REC_DT, isOutput=False)
    wi = nc.declare_dram_parameter("wi", [H, 4 * H], REC_DT, isOutput=False)
    wil = nc.declare_dram_parameter("wil", [H, 4 * H], REC_DT, isOutput=False)
    bg2 = nc.declare_dram_parameter("bg2", [128, 32], F32, isOutput=False)
    i32 = nc.declare_dram_parameter("i32", [32, 32], REC_DT, isOutput=False)
    wf = nc.declare_dram_parameter("wf", [H, OUT], F32, isOutput=False)
    bfr = nc.declare_dram_parameter("bfr", [B, OUT], F32, isOutput=False)
    out = nc.declare_dram_parameter("out", [B, OUT], F32, isOutput=True)

    KT = H // 128             # 2 k-tiles over the hidden dim
    MT = 4 * H // 128         # 8 m-tiles over the gate dim
    NB = KT * B               # 8: one gate's packed width
    W8 = 2 * NB               # 16
    Tanh = mybir.ActivationFunctionType.Tanh
    Sig = mybir.ActivationFunctionType.Sigmoid
    Exp = mybir.ActivationFunctionType.Exp
    ADD = mybir.AluOpType.add

    with tile.TileContext(nc) as tc:
        with (
            tc.tile_pool(name="const", bufs=1) as cp,
            tc.tile_pool(name="state", bufs=1) as sp,
            tc.tile_pool(name="acts", bufs=2) as ap_,
            tc.tile_pool(name="dve", bufs=2) as dp,
        ):
            # ---- constants / weights into SBUF -------------------------
            ident = cp.tile([128, 128], F32)
            make_identity(nc, ident)

            xt_sb = cp.tile([128, KT, B * SEQ], F32)    # x^T (numerator)
            nc.sync.dma_start(xt_sb, xt32f[:].rearrange("(k p) r -> p k r", p=128))
            xt16_0 = cp.tile([128, KT, B * SEQ], REC_DT)
            nc.sync.dma_start(xt16_0, xt16f[:].rearrange("(k p) r -> p k r", p=128))
            wax_sb = cp.tile([128, KT, H], REC_DT)
            nc.sync.dma_start(wax_sb, wax[:].rearrange("(k p) m -> p k m", p=128))
            wi_sb = cp.tile([128, KT, 4 * H], REC_DT)
            nc.sync.dma_start(wi_sb, wi[:].rearrange("(k p) m -> p k m", p=128))
            wil_sb = cp.tile([128, KT, 4 * H], REC_DT)
            nc.sync.dma_start(wil_sb, wil[:].rearrange("(k p) m -> p k m", p=128))
            wh_sb = cp.tile([128, KT, 4 * H], REC_DT)
            nc.sync.dma_start(wh_sb, wh[:].rearrange("(k p) m -> p k m", p=128))
            bg2_sb = cp.tile([128, 32], F32)
            nc.sync.dma_start(bg2_sb, bg2[:])
            i32_sb0 = cp.tile([32, 32], REC_DT)
            nc.sync.dma_start(i32_sb0, i32[:])
            wf_sb = cp.tile([128, KT, OUT], F32)
            nc.sync.dma_start(wf_sb, wf[:].rearrange("(k p) m -> p k m", p=128))
            bfr_sb = cp.tile([B, OUT], F32)
            nc.sync.dma_start(bfr_sb, bfr[:])

            # "Launder" matmul operands through a one-time DVE copy: matmul
            # waits lower into walrus's single-slot S3_LW struct, and mixing
            # a DMA-queue semaphore with a compute semaphore there is
            # rejected ("Too many sync wait commands").  After the copy,
            # matmuls only ever wait on the DVE semaphore.
            wax2 = cp.tile([128, KT, H], REC_DT)
            nc.vector.tensor_copy(
                wax2.rearrange("p a b -> p (a b)"),
                wax_sb.rearrange("p a b -> p (a b)"))
            wi2 = cp.tile([128, KT, 4 * H], REC_DT)
            nc.vector.tensor_copy(
                wi2.rearrange("p a b -> p (a b)"),
                wi_sb.rearrange("p a b -> p (a b)"))
            wil2 = cp.tile([128, KT, 4 * H], REC_DT)
            nc.vector.tensor_copy(
                wil2.rearrange("p a b -> p (a b)"),
                wil_sb.rearrange("p a b -> p (a b)"))
            wh2 = cp.tile([128, KT, 4 * H], REC_DT)
            nc.vector.tensor_copy(
                wh2.rearrange("p a b -> p (a b)"),
                wh_sb.rearrange("p a b -> p (a b)"))
            i32_sb = cp.tile([32, 32], REC_DT)
            nc.vector.tensor_copy(i32_sb, i32_sb0)
            wf2 = cp.tile([128, KT, OUT], F32)
            nc.vector.tensor_copy(
                wf2.rearrange("p a b -> p (a b)"),
                wf_sb.rearrange("p a b -> p (a b)"))

            with (
                tc.tile_pool(name="work", bufs=2) as wp,
                tc.tile_pool(name="ps_tr", bufs=2, space="PSUM") as ps_tr,
                tc.tile_pool(name="ps_s", bufs=2, space="PSUM") as ps_s,
            ):
                # ---- HAM warm-up: dummy matmuls make the PE clock gate
                # see sustained activity (K=8/8 -> 2.4 GHz); they only
                # depend on the identity, so they also absorb the gpsimd
                # wait before the first real transpose.
                pdum = ps_tr.tile([128, 64], F32, tag="dum", bufs=1)
                for _ in range(N_WARM_MM):
                    nc.tensor.matmul(pdum, ident, ident[:, 0:64],
                                     start=True, stop=True)

                # x^T comes pre-transposed from the host; launder the
                # fp16 copy (S-matmul rhs) through DVE.
                xt16 = cp.tile([128, KT, B * SEQ], REC_DT)
                nc.vector.tensor_copy(
                    xt16.rearrange("p a b -> p (a b)"),
                    xt16_0.rearrange("p a b -> p (a b)"))

                # ---- phase 2+3: scores, exp, weighted sums -------------
                # S^T = Wa_x^T @ x^T ; alpha-normalization is folded into
                # context = (sum_s E*x) / (sum_s E),  E = exp(S^T)
                ctx_sb = cp.tile([128, KT, B], REC_DT)   # context^T hi (G0 rhs)
                ctx32 = cp.tile([128, KT, B], F32)
                ctx_lo = cp.tile([128, KT, B], REC_DT)   # residual
                for m in range(KT):
                    ps = ps_s.tile([128, B * SEQ], F32)
                    for k in range(KT):
                        nc.tensor.matmul(
                            ps, wax2[:, k, m * 128:(m + 1) * 128],
                            xt16[:, k, :],
                            start=(k == 0), stop=(k == KT - 1),
                        )
                    e_sb = wp.tile([128, B, SEQ], F32, tag="e")
                    nc.scalar.activation(
                        e_sb.rearrange("p a b -> p (a b)"), ps, Exp)
                    p_sb = wp.tile([128, B, SEQ], F32, tag="p")
                    nc.vector.tensor_mul(
                        p_sb.rearrange("p a b -> p (a b)"),
                        e_sb.rearrange("p a b -> p (a b)"),
                        xt_sb[:, m, :],
                    )
                    den = dp.tile([128, B], F32, tag="den")
                    num = dp.tile([128, B], F32, tag="num")
                    nc.vector.tensor_reduce(
                        den, e_sb, axis=mybir.AxisListType.X, op=ADD)
                    nc.vector.tensor_reduce(
                        num, p_sb, axis=mybir.AxisListType.X, op=ADD)
                    rden = dp.tile([128, B], F32, tag="rden")
                    nc.vector.reciprocal(rden, den)
                    nc.vector.tensor_mul(ctx32[:, m, :], num, rden)
                    nc.vector.tensor_copy(ctx_sb[:, m, :], ctx32[:, m, :])
                    nc.vector.tensor_sub(
                        ctx_lo[:, m, :], ctx32[:, m, :], ctx_sb[:, m, :])

            # phase 1-3 PSUM pools are closed here, freeing their banks for
            # the recurrence pools below (stack allocator).
            with (
                tc.tile_pool(name="ps_g", bufs=2, space="PSUM") as ps_g,
                tc.tile_pool(name="ps_o", bufs=1, space="PSUM") as ps_o,
            ):
                # ---- phase 4: G0 = (context @ Wi + b)^T, packed --------
                psg0 = ps_g.tile([128, MT * B], F32, tag="psg_fi")
                for mt in range(MT):
                    for k in range(KT):
                        sl = psg0[:, mt * B:(mt + 1) * B]
                        whi = wi2[:, k, mt * 128:(mt + 1) * 128]
                        nc.tensor.matmul(sl, whi, ctx_sb[:, k, :],
                                         start=(k == 0), stop=False,
                                         skip_group_check=True)
                        nc.tensor.matmul(sl, whi, ctx_lo[:, k, :],
                                         start=False, stop=False,
                                         skip_group_check=True)
                        nc.tensor.matmul(
                            sl, wil2[:, k, mt * 128:(mt + 1) * 128],
                            ctx_sb[:, k, :],
                            start=False, stop=(k == KT - 1),
                            skip_group_check=True)
                g0_sb = cp.tile([128, MT * B], F32)
                nc.vector.tensor_add(g0_sb, psg0, bg2_sb)

                # G0^T slices (fp16) so each step's first matmuls write G0
                # into PSUM: out = (G0^T).T @ I = G0.   Split g / f,i / o
                # to match the three PSUM banks below.
                psg0t_g = ps_o.tile([NB, 128], F32, tag="g0t")
                nc.tensor.transpose(psg0t_g, g0_sb[:, 0:NB], ident)
                g0t_g = cp.tile([NB, 128], REC_DT)
                nc.vector.tensor_copy(g0t_g, psg0t_g)
                psg0t_fi = ps_o.tile([W8, 128], F32, tag="g0t")
                nc.tensor.transpose(psg0t_fi, g0_sb[:, NB:3 * NB], ident)
                g0t_fi = cp.tile([W8, 128], REC_DT)
                nc.vector.tensor_copy(g0t_fi, psg0t_fi)
                psg0t_o = ps_o.tile([NB, 128], F32, tag="g0t")
                nc.tensor.transpose(psg0t_o, g0_sb[:, 3 * NB:4 * NB], ident)
                g0t_o = cp.tile([NB, 128], REC_DT)
                nc.vector.tensor_copy(g0t_o, psg0t_o)

                # ---- phase 5: 64-step LSTM recurrence ------------------
                # gate cols (4 per block): g: 0:8 | f: 8:16, i: 16:24 |
                # o: 24:32, accumulated in three separate PSUM tiles
                # (banks) so tanh(g) fires after 5 matmuls and sig(f,i)
                # doesn't wait for the o matmuls.
                # state tile ctg = [ct | tanh(g)]: (128, 16)
                ctg = sp.tile([128, W8], F32)
                ht_sb = sp.tile([128, NB], REC_DT)

                for t in range(SEQ):
                    if t == 0:
                        gsrc_g = g0_sb[:, 0:NB]       # h0 = 0: gates = G0
                        gsrc_fi = g0_sb[:, NB:3 * NB]
                        gsrc_o = g0_sb[:, 3 * NB:4 * NB]
                    else:
                        psg_g = ps_g.tile([128, NB], F32, tag="psg_g")
                        psg_fi = ps_g.tile([128, W8], F32, tag="psg_fi")
                        psg_o = ps_g.tile([128, NB], F32, tag="psg_o")
                        nc.tensor.matmul(psg_g, g0t_g, i32_sb[0:NB, 0:NB],
                                         start=True, stop=False,
                                         skip_group_check=True)
                        for mt in range(2):
                            for k in range(KT):
                                nc.tensor.matmul(
                                    psg_g[:, mt * B:(mt + 1) * B],
                                    wh2[:, k, mt * 128:(mt + 1) * 128],
                                    ht_sb[:, k * B:(k + 1) * B],
                                    start=False, stop=(k == KT - 1),
                                    skip_group_check=True,
                                )
                        nc.tensor.matmul(
                            psg_fi, g0t_fi, i32_sb[0:W8, 0:W8],
                            start=True, stop=False, skip_group_check=True)
                        for mt in range(2, 6):
                            for k in range(KT):
                                nc.tensor.matmul(
                                    psg_fi[:, (mt - 2) * B:(mt - 1) * B],
                                    wh2[:, k, mt * 128:(mt + 1) * 128],
                                    ht_sb[:, k * B:(k + 1) * B],
                                    start=False, stop=(k == KT - 1),
                                    skip_group_check=True,
                                )
                        nc.tensor.matmul(
                            psg_o, g0t_o, i32_sb[0:NB, 0:NB],
                            start=True, stop=False, skip_group_check=True)
                        for mt in range(6, MT):
                            for k in range(KT):
                                nc.tensor.matmul(
                                    psg_o[:, (mt - 6) * B:(mt - 5) * B],
                                    wh2[:, k, mt * 128:(mt + 1) * 128],
                                    ht_sb[:, k * B:(k + 1) * B],
                                    start=False, stop=(k == KT - 1),
                                    skip_group_check=True,
                                )
                        gsrc_g = psg_g
                        gsrc_fi = psg_fi
                        gsrc_o = psg_o

                    # tanh(g) -> ctg[:, 8:16] (adjacent to ct)
                    nc.scalar.activation(ctg[:, NB:W8], gsrc_g, Tanh)
                    sfi = ap_.tile([128, W8], F32, tag="sfi")
                    nc.scalar.activation(sfi, gsrc_fi, Sig)
                    so = ap_.tile([128, NB], F32, tag="so")
                    nc.scalar.activation(so, gsrc_o, Sig)

                    if t == 0:
                        # ct = sig(i) * tanh(g)
                        nc.vector.tensor_mul(
                            ctg[:, 0:NB], sfi[:, NB:W8], ctg[:, NB:W8])
                    else:
                        # [av|bv] = [sig f|sig i] * [ct|tanh g] in one op
                        avbv = dp.tile([128, W8], F32, tag="avbv")
                        nc.vector.tensor_mul(avbv, sfi, ctg)
                        nc.vector.tensor_add(
                            ctg[:, 0:NB], avbv[:, 0:NB], avbv[:, NB:W8])

                    tc_ = ap_.tile([128, NB], F32, tag="tc")
                    nc.scalar.activation(tc_, ctg[:, 0:NB], Tanh)
                    if t < SEQ - 1:
                        nc.vector.tensor_mul(ht_sb, so, tc_)
                    else:
                        ht32 = sp.tile([128, NB], F32)
                        nc.vector.tensor_mul(ht32, so, tc_)

                # ---- phase 6: out = ht @ Wf + bf -----------------------
                pso = ps_o.tile([B, OUT], F32, tag="pso")
                for k in range(KT):
                    nc.tensor.matmul(
                        pso, ht32[:, k * B:(k + 1) * B], wf2[:, k, :],
                        start=(k == 0), stop=(k == KT - 1),
                    )
                out_sb = dp.tile([B, OUT], F32, tag="out")
                nc.vector.tensor_add(out_sb, pso, bfr_sb)
                nc.sync.dma_start(out[:], out_sb)

    nc.compile()
    return nc


_NC_CACHE = None


def _prep_common(Wa, Wi, Wh, b, Wf, bf):
    """Host-side weight prep shared across cores (all numpy, no device)."""
    Wa = np.asarray(Wa, np.float32)
    Wi = np.asarray(Wi, np.float32)
    Wh = np.asarray(Wh, np.float32)
    b = np.asarray(b, np.float32)
    Wf = np.asarray(Wf, np.float32)
    bf = np.asarray(bf, np.float32)

    # ht/ct rows of Wa (and ba) are constant along seq => cancel in softmax.
    wax = np.ascontiguousarray(Wa[:H].astype(REC_NP))

    # permute gate blocks to [g g f f i i o o]
    perm = np.concatenate([np.arange(mt * 128, (mt + 1) * 128)
                           for mt in GATE_PERM])
    wh_p = np.ascontiguousarray(Wh[:, perm].astype(REC_NP))
    wi_perm = Wi[:, perm]
    wi_p = np.ascontiguousarray(wi_perm.astype(REC_NP))
    wil_p = np.ascontiguousarray(
        (wi_perm - wi_p.astype(np.float32)).astype(REC_NP))
    b_p = b[perm]

    # bias packed: [partition p, (block j, batch b)]
    bg2 = np.ascontiguousarray(
        np.repeat(b_p.reshape(8, 128).T[:, :, None], B, axis=2).reshape(128, 32))
    i32 = np.ascontiguousarray(np.eye(32, dtype=REC_NP))
    bfr = np.ascontiguousarray(np.broadcast_to(bf.reshape(1, OUT), (B, OUT)))
    return {
        "wax": wax, "wh": wh_p, "wi": wi_p, "wil": wil_p,
        "bg2": bg2, "i32": i32,
        "wf": np.ascontiguousarray(Wf), "bfr": bfr,
    }


def _make_in_maps(x, common):
    x = np.ascontiguousarray(np.asarray(x, np.float32))
    in_maps = []
    for c in range(NCORES):
        xt = np.ascontiguousarray(x[c * B:(c + 1) * B].reshape(B * SEQ, H).T)
        in_maps.append({"xt32f": xt, "xt16f": xt.astype(REC_NP), **common})
    return in_maps


def kernel(x, Wa, ba, Wi, Wh, b, Wf, bf):
    """Full (unsharded) inputs -> full (32, 1) output."""
    global _NC_CACHE
    if _NC_CACHE is None:
        _NC_CACHE = _build_nc()
    common = _prep_common(Wa, Wi, Wh, b, Wf, bf)
    in_maps = _make_in_maps(x, common)
    res = run_bass_kernel_spmd(_NC_CACHE, in_maps, list(range(NCORES)))
    outs = [res.results[c]["out"] for c in range(NCORES)]
    return np.concatenate(outs, axis=0).astype(np.float32)
